# revision 16
# baseline (speedup 1.0000x reference)
"""MoE feed-forward (E=8 experts, top-2) for one TRN2 chip (8 NeuronCores).

Strategy: expert-parallel. Host computes the (tiny) router matmul + softmax
+ top-2 in numpy, gathers each expert's routed tokens, pads to a fixed
capacity C, and ships per-expert weights + gathered tokens to one core each.
Each core runs an identical Bass/Tile FFN program in bf16:

    GT = Wg^T @ X   (transposed-activation layout: [I, C] tiles)
    UT = Wu^T @ X
    AT = silu(GT) * UT          (bf16, SBUF-resident)
    YT = Wd^T_col-tiles @ AT    -> [H, C] bf16 out

The PE stream (~171us of bf16 matmul at C~1071) is the wall; the program
is organized to keep every non-PE cost off the critical path:
  - phase A runs token-chunk-major (chunk-0 pass over all 16 i-tiles,
    then chunk 1, then chunk 2), so the first matmuls need only 1/3 of x
    plus i-tile 0's gate/up weights, and each i-tile's weights arrive
    just ahead of first use;
  - phase B h-tile passes are interleaved between phase A passes
    (A-k0, A-k1, B-k0, A-k2, B-k1, B-k2) so the down-projection never
    waits on the tail of phase A, and output DMA is spread over the
    second half of the program;
  - input DMAs are issued from BOTH HWDGE queues (SP + Activation) in
    consumption order; a few dependency-free warmup matmuls bridge the
    tensor engine's fixed ~6.7us startup until the first pieces land;
  - per (i-tile, chunk) only one g/u PSUM pair is live and is consumed
    (silu+mul) while the next chunk's matmuls run — no PSUM-slot stalls;
  - y is written out in bf16 (error contribution ~0.2% of an output that
    has ~8x that from the bf16 matmuls), and the very last flush is 4
    partition-sliced DMAs across both queues to shrink the output tail.

The host applies the top-2 combine weights and scatters rows back into the
full [B, S, H] output.
"""

import numpy as np
import ml_dtypes

H = 1024
I = 2048
E = 8
TOPK = 2
P = 128
N_T = 3  # token chunks per core (chunk width C/3 <= 512 = one PSUM bank)

_PROGRAM_CACHE = {}
LAST_RESULT = None  # BassKernelResults of the most recent device run


def _build_program(C):
    from contextlib import ExitStack

    import concourse.mybir as mybir
    import concourse.tile as tile
    from concourse import bacc

    f32 = mybir.dt.float32
    bf16 = mybir.dt.bfloat16
    Silu = mybir.ActivationFunctionType.Silu

    n_h = H // P   # 8 contraction chunks over hidden dim
    n_i = I // P   # 16 tiles over intermediate dim
    NT = C // N_T  # token-chunk width
    assert C % N_T == 0 and NT <= 512

    nc = bacc.Bacc("TRN2", enable_partition_id=False)
    xT = nc.dram_tensor("xT", [H, C], bf16, kind="ExternalInput")
    # gate/up are host-prearranged to [p, i_tile, c, i_within] so each
    # per-i-tile DMA reads 2KB-contiguous lines per partition.
    wg = nc.dram_tensor("wg", [P, I // P, H // P, P], bf16, kind="ExternalInput")
    wu = nc.dram_tensor("wu", [P, I // P, H // P, P], bf16, kind="ExternalInput")
    wd = nc.dram_tensor("wd", [I, H], bf16, kind="ExternalInput")
    yT = nc.dram_tensor("yT", [H, C], bf16, kind="ExternalOutput")

    with tile.TileContext(nc) as tc:
        with ExitStack() as ctx:
            wpool = ctx.enter_context(tc.tile_pool(name="weights", bufs=1))
            atpool = ctx.enter_context(tc.tile_pool(name="atp", bufs=1))
            spool = ctx.enter_context(tc.tile_pool(name="stmp", bufs=4))
            ypool = ctx.enter_context(tc.tile_pool(name="yst", bufs=4))
            pspool = ctx.enter_context(
                tc.tile_pool(name="ps", bufs=8, space="PSUM")
            )

            # Short HAM warmup: dependency-free matmuls bridge the gap
            # between tensor-engine readiness (~6.7us) and the first
            # weight/x pieces landing (~8.5us), keeping the PE busy-window
            # continuous so the 2.4GHz clock arrives with the real stream.
            warm_src = wpool.tile([P, P], bf16, name="warm_src")
            nc.vector.memset(warm_src, 0.0)
            warm_ps = pspool.tile([P, NT], f32, tag="ps", name="warm_ps")

            def junk_mms(n):
                # Dependency-free matmuls that keep the PE busy while real
                # operands are still in flight. Any PE-idle window resets
                # the HAM activity monitor (>=3.4us re-throttles the clock
                # to 1.2GHz), so the warmup must abut the real stream, and
                # the first i-tile's sweep interleaves these between its
                # DMA-paced matmuls.
                for _ in range(n):
                    nc.tensor.matmul(
                        warm_ps[:, 0:P], warm_src, warm_src,
                        start=True, stop=True,
                    )

            junk_mms(30)

            x_s = wpool.tile([P, n_h, C], bf16, name="x_s")
            wg_s = wpool.tile([P, n_i, n_h, P], bf16, name="wg_s")
            wu_s = wpool.tile([P, n_i, n_h, P], bf16, name="wu_s")
            wd_s = wpool.tile([P, n_i, H], bf16, name="wd_s")
            at_s = atpool.tile([P, n_i, C], bf16, name="at_s")

            # Input DMAs. Data cannot land before ~9us (ring startup), and
            # each HWDGE queue is strict FIFO over its dma_starts at a
            # shared ~350GB/s — so ordering is everything. The ACT queue
            # carries only i-tiles 0/1's gate+up weights (4 issues; the
            # engine is then free for phase A's activations), while SP
            # streams x chunk 0, the remaining gate/up i-tiles, x chunks
            # 1-2, and wd, each just ahead of its first use in the k-major
            # schedule below.
            nc.scalar.dma_start(out=wg_s[:, 0, :, :], in_=wg[:, 0, :, :])
            nc.scalar.dma_start(out=wu_s[:, 0, :, :], in_=wu[:, 0, :, :])
            for c in range(n_h):
                nc.sync.dma_start(
                    out=x_s[:, c, 0:NT], in_=xT[c * P:(c + 1) * P, 0:NT]
                )
            for it in range(1, n_i):
                nc.sync.dma_start(out=wg_s[:, it, :, :], in_=wg[:, it, :, :])
                nc.sync.dma_start(out=wu_s[:, it, :, :], in_=wu[:, it, :, :])
            for c in range(n_h):
                nc.sync.dma_start(
                    out=x_s[:, c, NT:C], in_=xT[c * P:(c + 1) * P, NT:C]
                )
            for it in range(n_i):
                nc.sync.dma_start(
                    out=wd_s[:, it, :], in_=wd[it * P:(it + 1) * P, :]
                )

            def a_pass(k):
                # AT[:, it, chunk k] = silu(Wg^T X) * (Wu^T X) for all it
                for it in range(n_i):
                    g_ps = pspool.tile([P, NT], f32, tag="ps", name=f"g_{it}_{k}")
                    u_ps = pspool.tile([P, NT], f32, tag="ps", name=f"u_{it}_{k}")
                    # g-sweep before u-sweep: the silu on g_ps starts while
                    # the u-sweep still runs, and (for i-tile 0) wu's DMA
                    # gets an extra ~1.2us before its first use. The very
                    # first sweep is paced by the trickling x pieces
                    # (~0.5us apart vs 0.15us/matmul), so junk matmuls fill
                    # those gaps to keep the HAM clock monitor busy.
                    for c in range(n_h):
                        st, sp = (c == 0), (c == n_h - 1)
                        nc.tensor.matmul(
                            g_ps, wg_s[:, it, c, :],
                            x_s[:, c, k * NT:(k + 1) * NT], start=st, stop=sp,
                        )
                        if k == 0 and it == 0 and c < n_h - 1:
                            # taper matched to the early x-piece cadence
                            # (~1.2us between the first arrivals, narrowing
                            # as DMA bandwidth ramps)
                            junk_mms((10, 10, 6, 3, 2, 1, 0)[c])
                    for c in range(n_h):
                        st, sp = (c == 0), (c == n_h - 1)
                        nc.tensor.matmul(
                            u_ps, wu_s[:, it, c, :],
                            x_s[:, c, k * NT:(k + 1) * NT], start=st, stop=sp,
                        )
                    stile = spool.tile([P, NT], f32, tag="stmp", name=f"s_{it}_{k}")
                    nc.scalar.activation(stile, g_ps, Silu)
                    nc.vector.tensor_mul(
                        at_s[:, it, k * NT:(k + 1) * NT], stile, u_ps
                    )

            def b_pass(k, last=False):
                # YT[ht, chunk k] = sum_i Wd[i, ht]^T AT[i, chunk k]
                for ht in range(n_h):
                    y_ps = pspool.tile([P, NT], f32, tag="ps", name=f"y_{ht}_{k}")
                    for it in range(n_i):
                        st, sp = (it == 0), (it == n_i - 1)
                        nc.tensor.matmul(
                            y_ps, wd_s[:, it, ht * P:(ht + 1) * P],
                            at_s[:, it, k * NT:(k + 1) * NT], start=st, stop=sp,
                        )
                    yt = ypool.tile([P, NT], bf16, tag="yst", name=f"yo_{ht}_{k}")
                    nc.vector.tensor_copy(yt, y_ps)
                    if not (last and ht == n_h - 1):
                        nc.sync.dma_start(
                            out=yT[ht * P:(ht + 1) * P, k * NT:(k + 1) * NT],
                            in_=yt,
                        )
                    else:
                        # Final flush: 4 partition-sliced DMAs across both
                        # queues so the output tail is ~4 parallel ~12KB
                        # transfers instead of one serial 92KB one.
                        q = P // 4
                        for r in range(4):
                            eng = nc.sync if r % 2 == 0 else nc.scalar
                            eng.dma_start(
                                out=yT[ht * P + r * q:ht * P + (r + 1) * q,
                                       k * NT:(k + 1) * NT],
                                in_=yt[r * q:(r + 1) * q, :],
                            )

            a_pass(0)
            a_pass(1)
            b_pass(0)
            a_pass(2)
            b_pass(1)
            b_pass(2, last=True)

    nc.compile()
    return nc


def kernel(x, gate_w, wg, wu, wd):
    global LAST_RESULT
    x = np.asarray(x, dtype=np.float32)
    gate_w = np.asarray(gate_w, dtype=np.float32)
    wg = np.asarray(wg, dtype=np.float32)
    wu = np.asarray(wu, dtype=np.float32)
    wd = np.asarray(wd, dtype=np.float32)

    B, S, Hh = x.shape
    T = B * S
    xf = np.ascontiguousarray(x.reshape(T, Hh))

    # Router (tiny): logits -> softmax -> top-2, matching jax.lax.top_k
    # tie-order (stable sort prefers the lower expert index).
    logits = xf @ gate_w.T
    logits -= logits.max(axis=-1, keepdims=True)
    np.exp(logits, out=logits)
    probs = logits / logits.sum(axis=-1, keepdims=True)
    order = np.argsort(-probs, axis=1, kind="stable")[:, :TOPK]

    onehot = np.zeros((T, E), dtype=bool)
    onehot[np.arange(T)[:, None], order] = True
    tok_lists = [np.nonzero(onehot[:, e])[0] for e in range(E)]
    maxc = max(max(len(t) for t in tok_lists), N_T)
    C = int(-(-maxc // N_T) * N_T)  # round up to a multiple of N_T
    # PSUM (C/3 <= 512) and SBUF (x, AT, weights resident) cap C at ~1344;
    # the expected per-expert load is T*TOPK/E = 1024, so this is ample margin.
    assert C <= 1344, f"expert load too imbalanced for this kernel: {maxc}"

    nc = _PROGRAM_CACHE.get(C)
    if nc is None:
        nc = _build_program(C)
        _PROGRAM_CACHE[C] = nc

    bf = ml_dtypes.bfloat16
    xf_bf = xf.astype(bf)

    def _gu_layout(w):  # [H, I] -> [p, i_tile, c, j]
        return np.ascontiguousarray(
            w.reshape(H // P, P, I // P, P).transpose(1, 2, 0, 3)
        )

    in_maps = []
    for e in range(E):
        idx = tok_lists[e]
        xe = np.zeros((C, Hh), dtype=bf)
        xe[: len(idx)] = xf_bf[idx]
        in_maps.append(
            {
                "xT": np.ascontiguousarray(xe.T),
                "wg": _gu_layout(wg[e].astype(bf)),
                "wu": _gu_layout(wu[e].astype(bf)),
                "wd": wd[e].astype(bf),
            }
        )

    from concourse.bass_utils import run_bass_kernel_spmd

    res = run_bass_kernel_spmd(nc, in_maps, core_ids=list(range(E)))
    LAST_RESULT = res

    out = np.zeros((T, Hh), dtype=np.float32)
    for e in range(E):
        idx = tok_lists[e]
        ye = np.asarray(res.results[e]["yT"]).T[: len(idx)]
        out[idx] += probs[idx, e][:, None] * ye.astype(np.float32)
    return out.reshape(B, S, Hh)
